# revision 25
# baseline (speedup 1.0000x reference)
"""Trainium2 Bass kernel for a BAN (bilinear attention network) layer.

Reference computation (per batch b, head h, hd=64, scale=hd**-0.5):
    vp = (v @ Wv + bv)  -> [V=1024, 512] split into heads [h, V, 64]
    qp = (q @ Wq + bq)  -> [Q=512, 512]  split into heads [h, Q, 64]
    logits = vp_h @ att_w_h @ qp_h^T * scale        [V, Q]
    w = softmax(logits, axis=-1)
    pooled_v = mean_v(w @ qp_h)          [64]
    pooled_q = mean_q(w^T @ vp_h)        [64]
    fused = concat per head [pooled_v, pooled_q] -> [1024]
    out = relu(fused @ Wo + bo)          [512]

Algebraic simplifications (validated to ~4e-3 rel err vs ref):
  * rows of w sum to 1 => pooled_q = (1/Q) * colsum_v(vp_h)
  * pooled_v = z @ (q @ Wq)_h + bq_h with z = (1/V) sum_v e[v,:]/s[v]
    computed as a TensorE matmul with reciprocal rowsums stationary
  * att_w and the softmax scale are folded into Wq on the host.

Perf structure (v2):
  * exp+rowsum work is split between ScalarE (ACT Exp + accumulator,
    fp8 e) and VectorE (Schraudolph fast-exp: int16(l*A+B) bitcast to
    bf16; rowsums via one merged 3D tensor_reduce per head).
  * z colsums are col-tiled: 4 heads' M=1 matmuls run concurrently in
    one PSUM bank at array columns 0/32/64/96 (stationary broadcast to
    M=32 so the full bank is defined); drained with ONE [128,512] cast
    + an SBUF DMA restack instead of per-head [1,512] copies.
  * input DMA is split (critical tensors first) and the PE is kept warm
    through the DMA window with dummy matmuls so HAM reaches K=8/8
    before real work starts.

Sharding: data-parallel over batch, 2 batches per core, params
replicated, no collectives.  Host does only layout transforms.
"""

import numpy as np
import ml_dtypes

BF16 = ml_dtypes.bfloat16

B, V_NUM, Q_NUM = 16, 1024, 512
V_DIM, Q_DIM = 256, 128
HIDDEN, HEADS, HD = 512, 8, 64
SCALE = HD ** -0.5

N_CORES = 8
BPC = B // N_CORES          # batches per core
DC = V_DIM // 128           # d-chunks of v (2)
IB = HIDDEN // 128          # i-blocks of hidden (4)
QC = Q_NUM // 128           # q-chunks (4)
VB = V_NUM // 512           # v-blocks of 512 (2)
VCH = V_NUM // 128          # v-chunks of 128 (8)
NB = HIDDEN // 128          # out feature blocks (4)
KC = (2 * HEADS * HD) // 128  # fused feature chunks of 128 (8)

# Schraudolph fast-exp (bf16 bit domain), calibrated offline on the
# real logit distribution (trunc-to-int semantics): e^l ~= bf16 bits of
# int16(l * 2^7/ln2 + SCH_B).
SCH_A = 128.0 / float(np.log(2.0))
SCH_B = 16249.0
RB_SCALE = float(2.0 ** 18) / V_NUM
PV_SCALE = float(2.0 ** -18)

# chunks 0..SC-1 of each (b,h) drain on ScalarE (ACT exp, fp8 e),
# chunks SC..7 on VectorE (Schraudolph, bf16 e).  Alternate 5/6 per
# head to balance engine load; tuned against the HW trace.
SCE_CHUNKS = [6, 6, 5, 6, 6, 5, 6, 6]  # per head (same for both batches)
N_WARMUP_MM = 26

_CACHE = {}


def _build_nc():
    from contextlib import ExitStack

    import concourse.bass as bass
    import concourse.tile as tile
    from concourse import bacc, mybir
    from concourse.masks import make_identity
    from concourse.tile import add_dep_helper

    f32 = mybir.dt.float32
    bf16 = mybir.dt.bfloat16
    fp8 = mybir.dt.float8e4
    i16 = mybir.dt.int16
    AF = mybir.ActivationFunctionType
    ALU = mybir.AluOpType
    AX = mybir.AxisListType

    nc = bacc.Bacc("TRN2", target_bir_lowering=False)

    # packed0: data needed early (vt b0, qt both, Wv/Wqw/Wq, ident, biases)
    # packed1: vt b1 + Wo (epilogue)
    P0_VT = 0
    P0_QT = DC * V_NUM                      # 2048
    P0_WALL = P0_QT + BPC * Q_NUM           # +1024
    P0_ID = P0_WALL + 4 * HIDDEN            # wv(2)+wqw+wq blocks
    P0_BALL = P0_ID + 8
    P0_COLS = P0_BALL + 2 * (2 * IB + HEADS + NB)
    P1_VT = 0
    P1_WO = DC * V_NUM
    P1_COLS = P1_WO + KC * HIDDEN
    packed0_p = nc.declare_dram_parameter("packed0", [128, P0_COLS], bf16, isOutput=False)
    packed1_p = nc.declare_dram_parameter("packed1", [128, P1_COLS], bf16, isOutput=False)
    outT_p = nc.declare_dram_parameter("outT", [HIDDEN, BPC], f32, isOutput=True)

    with tile.TileContext(nc) as tc, ExitStack() as ctx:
        const = ctx.enter_context(tc.tile_pool(name="const", bufs=1))
        work = ctx.enter_context(tc.tile_pool(name="work", bufs=1))
        epool = ctx.enter_context(tc.tile_pool(name="epool", bufs=100))
        ipool = ctx.enter_context(tc.tile_pool(name="ipool", bufs=16))
        spool = ctx.enter_context(tc.tile_pool(name="spool", bufs=32))
        ps_big = ctx.enter_context(tc.tile_pool(name="ps_big", bufs=4, space="PSUM"))
        ps_z = ctx.enter_context(tc.tile_pool(name="ps_z", bufs=2, space="PSUM"))
        ps_sm = ctx.enter_context(tc.tile_pool(name="ps_sm", bufs=1, space="PSUM"))
        ps_w = ctx.enter_context(tc.tile_pool(name="ps_w", bufs=1, space="PSUM"))

        class SlotGuard:
            """Explicit WAR edges for psum slot reuse: the first writer of
            allocation i+bufs must wait for all readers of allocation i."""

            def __init__(self, bufs):
                self.bufs = bufs
                self.hist = []

            def alloc(self):
                self.hist.append([[], []])
                return len(self.hist) - 1

            def writer(self, idx, mi):
                if not self.hist[idx][0]:
                    prev = idx - self.bufs
                    if prev >= 0:
                        for r in self.hist[prev][1] + self.hist[prev][0]:
                            add_dep_helper(mi.ins, r.ins, sync=True,
                                           reason="psum slot WAR/WAW guard")
                self.hist[idx][0].append(mi)
                return mi

            def reader(self, idx, mi):
                self.hist[idx][1].append(mi)
                return mi

        GUARD_ON = True
        g_big = SlotGuard(4)
        g_z = SlotGuard(2)
        g_sm = SlotGuard(1)
        g_w = SlotGuard(1)

        # ---- warmup: keep the PE busy (and HAM un-throttled) through the
        # input-DMA window with dummy matmuls into a scratch psum bank ----
        warm_sb = const.tile([128, 512], bf16, tag="warm")
        nc.vector.memset(warm_sb[:], 0.0)
        # one psum bank shared by the HAM-keepalive zone (cols 0:480) and the
        # progressive-epilogue accumulators (cols 480:488).
        pw_all = ps_w.tile([128, 512], f32, name="pw", tag="w")
        pw_loop = pw_all[:, 0:480]
        for i in range(N_WARMUP_MM):
            nc.tensor.matmul(
                pw_loop, lhsT=warm_sb[:, 0:128], rhs=warm_sb[:, 0:480],
                start=(i == 0), stop=True)

        def ham_tick():
            """Zero-value matmul into the dead warmup region (start=False:
            no pending-zero side effects) to keep the PE array's HAM duty
            cycle high so real matmuls run at 2.4 GHz."""
            nc.tensor.matmul(
                pw_loop, lhsT=warm_sb[:, 0:128], rhs=warm_sb[:, 0:480],
                start=False, stop=False)

        # ---- inputs: two DMAs, critical data first ----
        packed0_sb = const.tile([128, P0_COLS], bf16, tag="p0")
        packed1_sb = const.tile([128, P1_COLS], bf16, tag="p1")
        nc.sync.dma_start(packed0_sb[:], packed0_p[:])
        nc.sync.dma_start(packed1_sb[:], packed1_p[:])

        vt0_sb = packed0_sb[:, P0_VT:P0_QT].rearrange("p (c v) -> p c v", c=DC)
        vt1_sb = packed1_sb[:, P1_VT:P1_WO].rearrange("p (c v) -> p c v", c=DC)
        vt_b = [vt0_sb, vt1_sb]
        qt_sb = packed0_sb[:, P0_QT:P0_WALL].rearrange("p (b q) -> p b q", b=BPC)
        wall_sb = packed0_sb[:, P0_WALL:P0_ID].rearrange("p (w h) -> p w h", w=4)
        wv_sb = wall_sb[:, 0:DC]
        wqw_sb = wall_sb[:, DC]
        wq_sb = wall_sb[:, DC + 1]
        wo_sb = packed1_sb[:, P1_WO:P1_COLS].rearrange("p (w h) -> p w h", w=KC)
        ident_sb = packed0_sb[0:8, P0_ID:P0_ID + 8]
        ball_sb = packed0_sb[:, P0_BALL:P0_COLS].bitcast(f32)
        bv_sb = ball_sb[:, 0:IB]
        bqw_sb = ball_sb[:, IB:2 * IB]
        fb_sb = ball_sb[:, 2 * IB:2 * IB + HEADS]
        bo_sb = ball_sb[:, 2 * IB + HEADS:]

        # ---- long-lived activations ----
        vpT_sb = work.tile([128, BPC, IB, V_NUM], bf16, tag="vpt")
        qpwT_sb = work.tile([128, BPC, IB, Q_NUM], bf16, tag="qpwt")
        qp_sb = work.tile([128, BPC, QC, HIDDEN], bf16, tag="qp")
        zfull_sb = work.tile([128, BPC, 2, Q_NUM], bf16, tag="zfull")
        zstack_sb = work.tile([4, BPC, 2, Q_NUM], bf16, tag="zstack")
        zT_sb = work.tile([128, BPC, QC, HEADS], bf16, tag="zT")
        fusedT_sb = work.tile([128, KC, BPC], bf16, tag="fused")
        outT_sb = work.tile([128, NB, BPC], f32, tag="outT")
        cv_sb = work.tile([128, BPC, DC], f32, tag="cv")
        cvb_sb = work.tile([128, BPC, DC], bf16, tag="cvb")

        # ---- prologue: projections.  Returns small thunks (one per psum
        # group) so the caller can spread them through the main loop. ----
        def prologue_thunks(b):
            thunks = []

            def th(fn):
                thunks.append(fn)
            for ib in range(IB):
                for vb in range(VB):
                    def vpt_group(ib=ib, vb=vb):
                        ps = ps_big.tile([128, 512], f32, tag="big")
                        gi = g_big.alloc()
                        for dc in range(DC):
                            g_big.writer(gi, nc.tensor.matmul(
                                ps[:], lhsT=wv_sb[:, dc, ib * 128:(ib + 1) * 128],
                                rhs=vt_b[b][:, dc, vb * 512:(vb + 1) * 512],
                                start=(dc == 0), stop=(dc == DC - 1)))
                        g_big.reader(gi, nc.vector.tensor_scalar_add(
                            vpT_sb[:, b, ib, vb * 512:(vb + 1) * 512], ps[:],
                            bv_sb[:, ib:ib + 1]))
                    th(vpt_group)

                def qpwt_group(ib=ib):
                    ps = ps_big.tile([128, 512], f32, tag="big")
                    gi = g_big.alloc()
                    g_big.writer(gi, nc.tensor.matmul(
                        ps[:], lhsT=wqw_sb[:, ib * 128:(ib + 1) * 128],
                        rhs=qt_sb[:, b, :], start=True, stop=True))
                    g_big.reader(gi, nc.vector.tensor_scalar_add(
                        qpwT_sb[:, b, ib, :], ps[:], bqw_sb[:, ib:ib + 1]))
                th(qpwt_group)
            for qc in range(QC):
                def qp_group(qc=qc):
                    ps = ps_big.tile([128, 512], f32, tag="big")
                    gi = g_big.alloc()
                    g_big.writer(gi, nc.tensor.matmul(
                        ps[:], lhsT=qt_sb[:, b, qc * 128:(qc + 1) * 128],
                        rhs=wq_sb[:], start=True, stop=True))
                    g_big.reader(gi, nc.vector.tensor_copy(
                        qp_sb[:, b, qc, :], ps[:]))
                th(qp_group)

            def cv_group():
                for dc in range(DC):
                    nc.vector.tensor_reduce(
                        cv_sb[:, b, dc:dc + 1], vt_b[b][:, dc, :],
                        axis=AX.X, op=ALU.add)
                    nc.vector.tensor_scalar_mul(
                        cvb_sb[:, b, dc:dc + 1], cv_sb[:, b, dc:dc + 1],
                        1.0 / Q_NUM)
            th(cv_group)
            for ib in range(IB):
                for half in range(2):
                    def pq_group(ib=ib, half=half):
                        h = 2 * ib + half
                        psq = ps_sm.tile([128, 8], f32, tag="sm")
                        gi = g_sm.alloc()
                        for dc in range(DC):
                            g_sm.writer(gi, nc.tensor.matmul(
                                psq[64:128, 0:1],
                                lhsT=wv_sb[:, dc, ib * 128 + 64 * half: ib * 128 + 64 * half + 64],
                                rhs=cvb_sb[:, b, dc:dc + 1],
                                start=(dc == 0), stop=(dc == DC - 1)))
                        g_sm.reader(gi, nc.vector.tensor_scalar_add(
                            fusedT_sb[64:128, h, b:b+1], psq[64:128, 0:1],
                            fb_sb[64:128, h:h + 1]))
                    th(pq_group)
            return thunks

        # ---- per-head softmax state ----
        class Head:
            def __init__(self, b, h):
                self.b, self.h = b, h
                self.sc = SCE_CHUNKS[h]
                self.nd = VCH - self.sc
                self.s = spool.tile([128, VCH], f32, tag="s")
                self.e8 = [None] * self.sc
                self.ei = (ipool.tile([128, max(self.nd, 1), 512], i16,
                                      name="ei", tag="ei")
                           if self.nd else None)
                self.rb8 = None
                self.rb16 = None

            def e_rhs(self, c):
                if c < self.sc:
                    return self.e8[c][:]
                return self.ei[:, c - self.sc, :].bitcast(bf16)

            def rb_col(self, c):
                if c < self.sc:
                    return self.rb8[:, c:c + 1]
                return self.rb16[:, c:c + 1]

        def drain_chunk(hd_, c, gi, ps):
            """Move exp(logits chunk) out of psum on the per-chunk engine."""
            if c < hd_.sc:
                e_t = epool.tile([128, 512], fp8, tag="e")
                hd_.e8[c] = e_t
                g_big.reader(gi, nc.scalar.activation(
                    e_t[:], ps[:], AF.Exp, accum_out=hd_.s[:, c:c + 1]))
            else:
                g_big.reader(gi, nc.vector.tensor_scalar(
                    hd_.ei[:, c - hd_.sc, :], ps[:], SCH_A, SCH_B,
                    ALU.mult, ALU.add))

        def finish_head_thunks(hd_):
            """After all 8 chunks: DVE rowsums, reciprocal, rb tiles —
            returned as thunks so the DVE work spreads through the next
            pair's stream instead of blocking it."""
            hd_.rb8 = (spool.tile([128, VCH], fp8, name="rb8", tag="rb8")
                       if hd_.sc else None)
            hd_.rb16 = (spool.tile([128, VCH], bf16, name="rb16", tag="rb16")
                        if hd_.nd else None)
            r_t = spool.tile([128, VCH], f32, name="r", tag="r")
            thunks = []
            if hd_.nd:
                def red():
                    nc.vector.tensor_reduce(
                        hd_.s[:, hd_.sc:], hd_.ei[:].bitcast(bf16),
                        axis=AX.X, op=ALU.add)
                thunks.append(red)

            def rec():
                nc.vector.reciprocal(r_t[:], hd_.s[:])
            thunks.append(rec)

            def rbs():
                if hd_.sc:
                    nc.vector.tensor_scalar_mul(
                        hd_.rb8[:, :hd_.sc], r_t[:, :hd_.sc], RB_SCALE)
                if hd_.nd:
                    nc.vector.tensor_scalar_mul(
                        hd_.rb16[:, hd_.sc:], r_t[:, hd_.sc:], RB_SCALE)
            thunks.append(rbs)
            return thunks

        # ---- z-group machinery: 4 heads col-tiled into one psum bank ----
        class ZGroup:
            def __init__(self, heads, g_idx):
                self.heads = heads           # list of 4 Head
                self.g_idx = g_idx           # 0/1 within the batch
                self.zps = ps_z.tile([128, 512], f32, name="zps", tag="z")
                self.gi = g_z.alloc()
                self.c = 0

            def emit_chunk(self):
                c = self.c
                for j, hd_ in enumerate(self.heads):
                    g_z.writer(self.gi, nc.tensor.matmul(
                        self.zps[32 * j:32 * j + 32, :],
                        lhsT=hd_.rb_col(c).broadcast_to([128, 32]),
                        rhs=hd_.e_rhs(c),
                        start=(c == 0), stop=(c == VCH - 1),
                        tile_position=(0, 32 * j)))
                self.c += 1
                return self.c == VCH

        def zdrain_thunks(grp):
            """After a group's z matmuls: drain bank, restack, transpose,
            pooled_v matmuls for its 4 heads."""
            b = grp.heads[0].b
            g_idx = grp.g_idx
            thunks = []

            def th(fn):
                thunks.append(fn)

            def drain():
                g_z.reader(grp.gi, nc.vector.tensor_copy(
                    zfull_sb[:, b, g_idx, :], grp.zps[:]))
                # gather rows {32j} -> zstack rows 0..3
                for j in range(4):
                    nc.sync.dma_start(
                        zstack_sb[j:j + 1, b, g_idx, :],
                        zfull_sb[32 * j:32 * j + 1, b, g_idx, :])
            th(drain)

            def transposes():
                for qc in range(QC):
                    pst = ps_sm.tile([128, 8], bf16, name="pst", tag="sm")
                    gi = g_sm.alloc()
                    g_sm.writer(gi, nc.tensor.transpose(
                        pst[:, 0:4],
                        zstack_sb[0:4, b, g_idx, qc * 128:(qc + 1) * 128],
                        ident_sb[0:4, 0:4]))
                    g_sm.reader(gi, nc.vector.tensor_copy(
                        zT_sb[:, b, qc, 4 * g_idx:4 * g_idx + 4], pst[:, 0:4]))
            th(transposes)
            for hd_ in grp.heads:
                def pv_group(hd_=hd_):
                    h = hd_.h
                    psv = ps_sm.tile([128, 8], f32, tag="sm")
                    gi = g_sm.alloc()
                    for qc in range(QC):
                        g_sm.writer(gi, nc.tensor.matmul(
                            psv[0:64, 0:1],
                            lhsT=qp_sb[:, b, qc, h * 64:(h + 1) * 64],
                            rhs=zT_sb[:, b, qc, h:h + 1],
                            start=(qc == 0), stop=(qc == QC - 1)))
                    g_sm.reader(gi, nc.vector.tensor_scalar(
                        fusedT_sb[0:64, h, b:b+1], psv[0:64, 0:1],
                        PV_SCALE, fb_sb[0:64, h:h + 1],
                        ALU.mult, ALU.add))
                    if b == 1:
                        epilogue_feed(hd_.h)
                th(pv_group)
            return thunks

        # ---- progressive epilogue: out = relu(fused @ Wo + bo).
        # pso chains accumulate over kc; head h's kc-matmuls are emitted as
        # soon as batch-1's pv/pq for that head are done, so only the last
        # head's matmuls sit in the tail. ----
        pso_eps = []

        def epilogue_start():
            pso = pw_all[:, 480:480 + NB * BPC].rearrange(
                "p (n b) -> p n b", n=NB)
            pso_eps.append((pso, g_w.alloc()))

        def epilogue_feed(kc):
            # one accumulation group for all NB sub-chains: the kc==0,nb==0
            # start pending-zeroes the whole per-partition region, so each
            # sub-chain's first write overwrites and later ones accumulate.
            pso, gw = pso_eps[0]
            for nb in range(NB):
                g_w.writer(gw, nc.tensor.matmul(
                    pso[:, nb, :],
                    lhsT=wo_sb[:, kc, nb * 128:(nb + 1) * 128],
                    rhs=fusedT_sb[:, kc, :],
                    start=(kc == 0 and nb == 0),
                    stop=(kc == KC - 1 and nb == NB - 1)))

        def epilogue_finish():
            pso, gw = pso_eps[0]
            for nb in range(NB):
                g_w.reader(gw, nc.scalar.activation(
                    outT_sb[:, nb, :], pso[:, nb, :], AF.Relu,
                    bias=bo_sb[:, nb:nb + 1]))
                nc.sync.dma_start(
                    outT_p[:].rearrange("(o p) b -> p o b", p=128)[:, nb, :],
                    outT_sb[:, nb, :])

        # ---- main loop ----
        def emit_batch(b, pre_work, carry_grp=None):
            """Emits pairs of heads; z groups (incl. the previous batch's
            carry) are interleaved chunk-by-chunk; pre_work thunks fill PE
            idle slots.  Returns the trailing z-group."""
            heads = [Head(b, h) for h in range(HEADS)]
            pending_grp = carry_grp  # ZGroup currently streaming its chunks

            for t in range(HEADS // 2):
                hA, hB = heads[2 * t], heads[2 * t + 1]
                for c in range(VCH):
                    for side, hb in ((hA, 0), (hB, 64)):
                        ps = ps_big.tile([128, 512], f32, tag="big")
                        gi = g_big.alloc()
                        g_big.writer(gi, nc.tensor.matmul(
                            ps[:],
                            lhsT=vpT_sb[hb:hb + 64, b, t, c * 128:(c + 1) * 128],
                            rhs=qpwT_sb[hb:hb + 64, b, t, :],
                            start=True, stop=True))
                        drain_chunk(side, c, gi, ps)
                    for _ in range(2):
                        if pre_work:
                            pre_work.pop(0)()
                    if pending_grp is not None and c >= 3:
                        done = pending_grp.emit_chunk()
                        if not done and c in (3, 4, 5):
                            done = pending_grp.emit_chunk()
                        if done:
                            pre_work[0:0] = zdrain_thunks(pending_grp)
                            pending_grp = None
                    if c % 2 == 0:
                        ham_tick()
                pre_work[0:0] = finish_head_thunks(hA) + finish_head_thunks(hB)
                ham_tick()
                ham_tick()
                if t % 2 == 1:
                    # group of 4 heads complete -> start its z matmuls
                    grp = ZGroup(heads[2 * t - 2:2 * t + 2], (t - 1) // 2)
                    if pending_grp is not None:
                        while not pending_grp.emit_chunk():
                            pass
                        pre_work[0:0] = zdrain_thunks(pending_grp)
                    pending_grp = grp
            while pre_work:
                pre_work.pop(0)()
            return pending_grp

        pro0 = prologue_thunks(0)
        for fn in pro0[:3]:
            fn()
        epilogue_start()
        tail0 = emit_batch(0, pro0[3:] + prologue_thunks(1))
        tail1 = emit_batch(1, [], carry_grp=tail0)
        if tail1 is not None:
            while not tail1.emit_chunk():
                pass
            for fn in zdrain_thunks(tail1):
                fn()
        epilogue_finish()

    nc.compile()
    return nc


def _get_nc():
    if "nc" not in _CACHE:
        _CACHE["nc"] = _build_nc()
    return _CACHE["nc"]


def _host_prep(v, q, Wv, bv, Wq, bq, att_w, Wo, bo):
    """Host-side layout transforms + weight folding. Returns per-core in_maps."""
    v = np.asarray(v, np.float32)
    q = np.asarray(q, np.float32)
    Wv = np.asarray(Wv, np.float32)
    bv = np.asarray(bv, np.float32)
    Wq = np.asarray(Wq, np.float32)
    bq = np.asarray(bq, np.float32)
    att_w = np.asarray(att_w, np.float32)
    Wo = np.asarray(Wo, np.float32)
    bo = np.asarray(bo, np.float32)

    # fold att_w and softmax scale into the q projection
    Wq_h = Wq.reshape(Q_DIM, HEADS, HD)
    Wqw = (SCALE * np.einsum("dhj,hij->dhi", Wq_h, att_w)).reshape(Q_DIM, HIDDEN)
    bqw = (SCALE * np.einsum("hj,hij->hi", bq.reshape(HEADS, HD), att_w)).reshape(HIDDEN)

    wall = np.concatenate([
        Wv.reshape(DC, 128, HIDDEN).transpose(1, 0, 2),
        Wqw.reshape(1, 128, HIDDEN).transpose(1, 0, 2),
        Wq.reshape(1, 128, HIDDEN).transpose(1, 0, 2),
    ], axis=1).reshape(128, 4 * HIDDEN)
    wo_cols = Wo.reshape(KC, 128, HIDDEN).transpose(1, 0, 2).reshape(128, KC * HIDDEN)
    fbias = np.concatenate(
        [bq.reshape(HEADS, HD).T,
         (V_NUM / Q_NUM) * bv.reshape(HEADS, HD).T], axis=0)
    ball = np.concatenate([
        bv.reshape(IB, 128).T, bqw.reshape(IB, 128).T,
        fbias, bo.reshape(NB, 128).T], axis=1).astype(np.float32)
    ident = np.zeros((128, 8), np.float32)
    ident[:8, :8] = np.eye(8)
    shared0_cols = np.concatenate([
        wall.astype(BF16), ident.astype(BF16),
        np.ascontiguousarray(ball).view(BF16)], axis=1)
    in_maps = []
    for i in range(N_CORES):
        sl = slice(i * BPC, (i + 1) * BPC)
        vt = v[sl].transpose(0, 2, 1).reshape(BPC, DC, 128, V_NUM)
        vt = vt.transpose(2, 0, 1, 3).reshape(128, BPC, DC * V_NUM)
        qt = q[sl].transpose(0, 2, 1).transpose(1, 0, 2).reshape(128, BPC * Q_NUM)
        packed0 = np.concatenate(
            [vt[:, 0].astype(BF16), qt.astype(BF16), shared0_cols], axis=1)
        packed1 = np.concatenate(
            [vt[:, 1].astype(BF16), wo_cols.astype(BF16)], axis=1)
        in_maps.append({"packed0": np.ascontiguousarray(packed0),
                        "packed1": np.ascontiguousarray(packed1)})
    return in_maps


def kernel(**inputs):
    from concourse.bass_utils import run_bass_kernel_spmd

    nc = _get_nc()
    in_maps = _host_prep(**inputs)
    res = run_bass_kernel_spmd(nc, in_maps, core_ids=list(range(N_CORES)))
    out = np.empty((B, HIDDEN), np.float32)
    for i in range(N_CORES):
        out[i * BPC:(i + 1) * BPC] = np.asarray(res.results[i]["outT"]).T
    return out
